# revision 13
# baseline (speedup 1.0000x reference)
"""GQA attention prefill kernel for Trainium2 (Bass/Tile), 8-way tensor
parallel over heads.

Problem (hardcoded): B=1, S=2048, HID=4096, NH=32, KVH=8, D=128, causal
prefill with per-head RMSNorm on q/k and RoPE, positions = arange(S).

Sharding: core c owns kv-head c and q-heads 4c..4c+3. wq/wo sharded on the
head dim, wk/wv on the kv-head dim; x, rope tables replicated. Each core
computes its 4 heads' contribution through wo; the host sums the 8 partial
outputs.

Device pipeline (all matmul operands bf16, PSUM accumulation fp32):
- weights + rope tables resident in SBUF, loaded once with big DMAs; x
  streamed per 512-column chunk in two halves; y written per 128-row block.
- per chunk: kv projection -> V transpose -> q projection -> rms-norm
  factors via ones-matmul column sums -> rope (rotate-half via SBUF
  partition-swap DMAs) -> causal attention (score/exp/dn/av pipelined
  per k-tile) -> output projection (delayed one chunk so its matmuls fill
  the PE while the next chunk's norm/rope chains run).
- softmax/no-max-subtraction is safe: |scores| <= sqrt(128) after rms-norm.
- reciprocals use the fast DVE Newton approximation (~18 bits, 5x faster
  than the exact one); rms rsqrt = Sqrt activation + fast reciprocal.

Host side: inputs are pre-transposed/pre-reshaped to the exact SBUF layouts
(contraction dim on partitions) and cast to bf16; per-head norm weights and
the rotate-half sign are folded into the rope tables; outputs come back as
bf16 partials summed on host in fp32.
"""

import numpy as np
import ml_dtypes

import concourse.bass as bass
import concourse.mybir as mybir
import concourse.tile as tile
from concourse import bacc
from concourse.masks import make_identity

P = 128
S = 2048
HID = 4096
D = 128
G = 4            # q heads per core
NHT = HID // P   # 32 h-tiles (contraction)
SC = 512         # seq chunk
NSC = S // SC    # 4
NKT = S // P     # 16 k-tiles
EPS = 1e-6
N_CORES = 8

F32 = mybir.dt.float32
BF16 = mybir.dt.bfloat16
BF16_NP = ml_dtypes.bfloat16


def build_program():
    nc = bacc.Bacc("TRN2", target_bir_lowering=False, debug=False)

    # all device tensors are pre-arranged on host to the SBUF layout
    x_d = nc.dram_tensor("x_d", [P, NHT, S], BF16, kind="ExternalInput").ap()
    wq_d = nc.dram_tensor("wq_d", [P, NHT, G * P], BF16, kind="ExternalInput").ap()
    wk_d = nc.dram_tensor("wk_d", [P, NHT, P], BF16, kind="ExternalInput").ap()
    wv_d = nc.dram_tensor("wv_d", [P, NHT, P], BF16, kind="ExternalInput").ap()
    wo_d = nc.dram_tensor("wo_d", [P, G, HID], BF16, kind="ExternalInput").ap()
    cosq = nc.dram_tensor("cosq", [D, S], BF16, kind="ExternalInput").ap()
    sinq = nc.dram_tensor("sinq", [D, S], BF16, kind="ExternalInput").ap()
    cosk = nc.dram_tensor("cosk", [D, S], BF16, kind="ExternalInput").ap()
    sink = nc.dram_tensor("sink", [D, S], BF16, kind="ExternalInput").ap()
    y = nc.dram_tensor("y", [S, HID], BF16, kind="ExternalOutput").ap()

    Sqrt = mybir.ActivationFunctionType.Sqrt
    Exp = mybir.ActivationFunctionType.Exp
    Square = mybir.ActivationFunctionType.Square

    with tile.TileContext(nc) as tc:
        with (
            tc.tile_pool(name="const", bufs=1) as const,
            tc.tile_pool(name="xp", bufs=1) as xp,
            tc.tile_pool(name="scr", bufs=2) as scr,
            tc.tile_pool(name="qrp", bufs=4) as qrp,
            tc.tile_pool(name="ptp", bufs=3) as ptp,
            tc.tile_pool(name="otp", bufs=8) as otp,
            tc.tile_pool(name="ysp", bufs=2) as ysp,
            tc.tile_pool(name="psA", bufs=4, space="PSUM") as psA,
            tc.tile_pool(name="psB", bufs=2, space="PSUM") as psB,
            tc.tile_pool(name="psC", bufs=1, space="PSUM") as psC,
        ):
            # ---- first-chunk x + weight loads, split so the PE can start
            # on the first k/v h-tiles after ~1.5 MB instead of ~12 MB ----
            xa = [None] * NSC
            xb = [None] * NSC
            wk_sb = const.tile([P, NHT, P], BF16)
            wv_sb = const.tile([P, NHT, P], BF16)
            xa[0] = xp.tile([P, NHT // 2, SC], BF16, tag="xa", bufs=2, name="xa0")
            nc.sync.dma_start(xa[0][:, 0:8, :], x_d[:, 0:8, 0:SC])
            nc.sync.dma_start(wk_sb[:, 0:8, :], wk_d[:, 0:8, :])
            nc.sync.dma_start(wv_sb[:, 0:8, :], wv_d[:, 0:8, :])
            nc.sync.dma_start(xa[0][:, 8:16, :], x_d[:, 8:16, 0:SC])
            nc.sync.dma_start(wk_sb[:, 8:16, :], wk_d[:, 8:16, :])
            nc.sync.dma_start(wv_sb[:, 8:16, :], wv_d[:, 8:16, :])
            xb[0] = xp.tile([P, NHT // 2, SC], BF16, tag="xb", bufs=1, name="xb0")
            nc.sync.dma_start(xb[0][:, 0:8, :], x_d[:, 16:24, 0:SC])
            nc.sync.dma_start(wk_sb[:, 16:32, :], wk_d[:, 16:32, :])
            nc.sync.dma_start(wv_sb[:, 16:32, :], wv_d[:, 16:32, :])
            nc.sync.dma_start(xb[0][:, 8:16, :], x_d[:, 24:32, 0:SC])
            wq_sb = const.tile([P, NHT, G * P], BF16)
            for i in range(4):
                nc.sync.dma_start(wq_sb[:, 8 * i:8 * (i + 1), :],
                                  wq_d[:, 8 * i:8 * (i + 1), :])
            ck_sb = const.tile([D, S], BF16)
            nc.sync.dma_start(ck_sb, cosk)
            sk_sb = const.tile([D, S], BF16)
            nc.sync.dma_start(sk_sb, sink)
            cq_sb = const.tile([D, S], BF16)
            nc.sync.dma_start(cq_sb, cosq)
            sq_sb = const.tile([D, S], BF16)
            nc.sync.dma_start(sq_sb, sinq)
            wo_sb = const.tile([P, G, HID], BF16)
            nc.sync.dma_start(wo_sb[:, 0:2, :], wo_d[:, 0:2, :])
            nc.sync.dma_start(wo_sb[:, 2:4, :], wo_d[:, 2:4, :])

            # ---- constants ----
            f32tmp = const.tile([P, SC], F32)
            identity = const.tile([P, P], BF16)
            make_identity(nc, f32tmp[:, 0:P])
            nc.vector.tensor_copy(identity, f32tmp[:, 0:P])
            # ones[k, m] == 1: matmul(out, ones, rhs) -> column sums of rhs
            # broadcast across all 128 output partitions.
            ones_bf = const.tile([P, P], BF16)
            nc.gpsimd.memset(f32tmp, 1.0)
            nc.vector.tensor_copy(ones_bf, f32tmp[:, 0:P])
            # causal masks for the 4 diagonal k-tiles of a q chunk:
            # keep (1.0) where q_local >= 128*j + k_local
            masks = []
            for j in range(4):
                mk = const.tile([P, SC], BF16, name=f"mask{j}")
                nc.gpsimd.memset(f32tmp, 1.0)
                nc.gpsimd.affine_select(
                    f32tmp, f32tmp, pattern=[[1, SC]],
                    compare_op=mybir.AluOpType.is_ge,
                    fill=0.0, base=-P * j, channel_multiplier=-1,
                )
                nc.vector.tensor_copy(mk, f32tmp)
                masks.append(mk)

            bias_keps = const.tile([P, 1], F32)
            nc.gpsimd.memset(bias_keps, float(P) * EPS)
            bias_qeps = const.tile([P, 1], F32)
            nc.gpsimd.memset(bias_qeps, EPS)

            KR = const.tile([P, S], BF16)       # roped+scaled K, [d, s]
            Vs = const.tile([P, NKT, P], BF16)  # V, [s-in-tile, k-tile, d]

            # ots[sc][h]: attention outputs, consumed by the (delayed) o-proj
            ots = [[None] * G for _ in range(NSC)]

            def oproj_gen(sc):
                """output projection for chunk sc, as a generator yielding
                once per matmul so attention(sc+1) can interleave it into
                the PE stream to fill the ACT-exp stalls. Evacuations run
                on DVE (ACT is exp-bound during attention)."""
                q0 = sc * SC
                for stl in range(SC // P):
                    srow = q0 + stl * P
                    for grp in range(2):
                        ys = ysp.tile([P, HID // 2], BF16, tag="ys")
                        yps_l = [psA.tile([P, SC], F32, tag="acc",
                                          name=f"yps{j}") for j in range(4)]
                        for h in range(G):
                            lhs = ots[sc][h][:, stl * P:(stl + 1) * P]
                            for j in range(4):
                                hc = grp * 4 + j
                                nc.tensor.matmul(
                                    yps_l[j], lhs,
                                    wo_sb[:, h, hc * SC:(hc + 1) * SC],
                                    start=(h == 0), stop=(h == G - 1),
                                )
                                yield
                        for j in range(4):
                            if j % 2 == 0:
                                nc.vector.tensor_copy(
                                    ys[:, j * SC:(j + 1) * SC], yps_l[j])
                            else:
                                nc.scalar.copy(
                                    ys[:, j * SC:(j + 1) * SC], yps_l[j])
                        nc.sync.dma_start(
                            y[srow:srow + P,
                              grp * (HID // 2):(grp + 1) * (HID // 2)], ys)

            def drain(gen, n):
                for _ in range(n):
                    try:
                        next(gen)
                    except StopIteration:
                        return

            for sc in range(NSC):
                q0 = sc * SC

                # prefetch next chunk's x
                if sc + 1 < NSC:
                    q1 = (sc + 1) * SC
                    xa[sc + 1] = xp.tile([P, NHT // 2, SC], BF16, tag="xa",
                                         bufs=2, name=f"xa{sc + 1}")
                    nc.sync.dma_start(xa[sc + 1], x_d[:, 0:16, q1:q1 + SC])
                    xb[sc + 1] = xp.tile([P, NHT // 2, SC], BF16, tag="xb",
                                         bufs=1, name=f"xb{sc + 1}")
                    nc.sync.dma_start(xb[sc + 1], x_d[:, 16:32, q1:q1 + SC])

                def xt(ht):
                    return (xa[sc][:, ht, :] if ht < 16
                            else xb[sc][:, ht - 16, :])

                # ---- K/V projections ----
                kps = psA.tile([P, SC], F32, tag="acc")
                vps = psA.tile([P, SC], F32, tag="acc")
                for ht in range(NHT):
                    st, sp = ht == 0, ht == NHT - 1
                    nc.tensor.matmul(kps, wk_sb[:, ht, :], xt(ht),
                                     start=st, stop=sp)
                    nc.tensor.matmul(vps, wv_sb[:, ht, :], xt(ht),
                                     start=st, stop=sp)

                # ---- V: transpose [d, s] -> [s, d] tiles ----
                vtmp = scr.tile([P, SC], BF16, tag="vtmp")
                nc.vector.tensor_copy(vtmp, vps)
                for j in range(SC // P):
                    tp = psB.tile([P, P], BF16, tag="pt", name=f"tp{j}")
                    nc.tensor.transpose(tp, vtmp[:, j * P:(j + 1) * P],
                                        identity)
                    nc.vector.tensor_copy(Vs[:, sc * 4 + j, :], tp)

                # ---- K: sum-of-squares on ACT while q-proj runs on PE ----
                sqk = scr.tile([P, SC], BF16, tag="sqk")
                nc.scalar.activation(sqk, kps, Square)
                kraw = scr.tile([P, SC], BF16, tag="kraw")
                nc.vector.tensor_copy(kraw, kps)

                # ---- Q projections ----
                qps = [psA.tile([P, SC], F32, tag="acc", name=f"qps{i}")
                       for i in range(G)]
                for ht in range(NHT):
                    st, sp = ht == 0, ht == NHT - 1
                    for mt in range(G):
                        nc.tensor.matmul(
                            qps[mt], wq_sb[:, ht, mt * P:(mt + 1) * P],
                            xt(ht), start=st, stop=sp,
                        )

                # ---- K: norm factor + rope -> KR columns ----
                ssb = psB.tile([P, SC], F32, tag="pt")
                nc.tensor.matmul(ssb, ones_bf, sqk, start=True, stop=True)
                # 1/sqrt(ssq + d*eps) == rsqrt(mean+eps)/sqrt(d):
                # k-norm and softmax 1/sqrt(d) in one factor
                rkf = scr.tile([P, SC], F32, tag="rkf")
                nc.scalar.activation(rkf, ssb, Sqrt, bias=bias_keps, scale=1.0)
                rkr = scr.tile([P, SC], F32, tag="rkr")
                nc.vector.reciprocal_approx_fast(rkr, rkf)
                # rope: out = z*cos + rot(z)*sin_eff, rot(z) = [z2; z1]
                # (rotate-half minus sign folded into sin_eff on host)
                krot = scr.tile([P, SC], BF16, tag="krot")
                nc.sync.dma_start(krot[0:64], kraw[64:128])
                nc.sync.dma_start(krot[64:128], kraw[0:64])
                t1 = scr.tile([P, SC], BF16, tag="t1")
                nc.vector.tensor_mul(t1, krot, sk_sb[:, q0:q0 + SC])
                kpre = scr.tile([P, SC], BF16, tag="kpre")
                nc.vector.tensor_mul(kpre, kraw, ck_sb[:, q0:q0 + SC])
                nc.vector.tensor_add(kpre, kpre, t1)
                nc.vector.tensor_mul(KR[:, q0:q0 + SC], kpre, rkr)

                # ---- Q per head: norm factor + rope ----
                qrs = []
                for h in range(G):
                    sqq = scr.tile([P, SC], BF16, tag="sqk")
                    nc.scalar.activation(sqq, qps[h], Square)
                    ssbq = psB.tile([P, SC], F32, tag="pt")
                    nc.tensor.matmul(ssbq, ones_bf, sqq,
                                     start=True, stop=True)
                    rqf = scr.tile([P, SC], F32, tag="rkf")
                    nc.scalar.activation(rqf, ssbq, Sqrt,
                                         bias=bias_qeps, scale=1.0 / P)
                    rqr = scr.tile([P, SC], F32, tag="rkr")
                    nc.vector.reciprocal_approx_fast(rqr, rqf)
                    qraw = scr.tile([P, SC], BF16, tag="kraw")
                    nc.vector.tensor_copy(qraw, qps[h])
                    qrot = scr.tile([P, SC], BF16, tag="krot")
                    nc.sync.dma_start(qrot[0:64], qraw[64:128])
                    nc.sync.dma_start(qrot[64:128], qraw[0:64])
                    t1b = scr.tile([P, SC], BF16, tag="t1")
                    nc.vector.tensor_mul(t1b, qrot, sq_sb[:, q0:q0 + SC])
                    qpre = scr.tile([P, SC], BF16, tag="kpre")
                    nc.vector.tensor_mul(qpre, qraw, cq_sb[:, q0:q0 + SC])
                    nc.vector.tensor_add(qpre, qpre, t1b)
                    qr = qrp.tile([P, SC], BF16, tag="qr")
                    nc.vector.tensor_mul(qr, qpre, rqr)
                    qrs.append(qr)

                # ---- attention; o-proj of the previous chunk interleaves
                # into the PE stream to cover the ACT exp latency ----
                filler = oproj_gen(sc - 1) if sc > 0 else iter(())
                nkt = (sc + 1) * 4

                def qoff(kt):
                    # diagonal k-tile j attends only to q >= j*128 (causal)
                    return max(0, kt - sc * 4) * P

                for h in range(G):
                    avp = psC.tile([P, SC], F32, tag="av")
                    acc = scr.tile([P, SC], BF16, tag="acc")
                    ptps = [None] * nkt
                    ptps[0] = psB.tile([P, SC], F32, tag="pt", name="ptps0")
                    nc.tensor.matmul(ptps[0], KR[:, 0:P], qrs[h][:, qoff(0):],
                                     start=True, stop=True)
                    for kt in range(nkt):
                        o = qoff(kt)
                        if kt + 1 < nkt:
                            k1 = (kt + 1) * P
                            o1 = qoff(kt + 1)
                            ptps[kt + 1] = psB.tile([P, SC], F32, tag="pt",
                                                    name=f"ptps{kt + 1}")
                            nc.tensor.matmul(ptps[kt + 1][:, o1:],
                                             KR[:, k1:k1 + P],
                                             qrs[h][:, o1:],
                                             start=True, stop=True)
                        pt = ptp.tile([P, SC], BF16, tag="pt")
                        nc.scalar.activation(pt[:, o:], ptps[kt][:, o:], Exp)
                        if kt >= sc * 4:
                            # triangle mask on the partial 128-wide block
                            nc.vector.tensor_mul(pt[:, o:o + P], pt[:, o:o + P],
                                                 masks[0][:, 0:P])
                        # denominator: accumulate on idle GpSimd, one matmul
                        # (for the cross-partition sum + broadcast) at end
                        if kt == 0:
                            nc.gpsimd.tensor_copy(acc, pt)
                        else:
                            nc.gpsimd.tensor_add(acc[:, o:], acc[:, o:],
                                                 pt[:, o:])
                        nc.tensor.matmul(avp[:, o:], Vs[:, kt, :], pt[:, o:],
                                         start=(kt == 0), stop=(kt == nkt - 1))
                        drain(filler, 3)
                    dnp = psC.tile([P, SC], F32, tag="dn")
                    nc.tensor.matmul(dnp, ones_bf, acc, start=True, stop=True)
                    rcp = scr.tile([P, SC], F32, tag="rcp")
                    nc.vector.reciprocal_approx_fast(rcp, dnp)
                    ot = otp.tile([P, SC], BF16, tag="ot")
                    nc.vector.tensor_mul(ot, avp, rcp)
                    ots[sc][h] = ot
                drain(filler, 10 ** 6)

            drain(oproj_gen(NSC - 1), 10 ** 6)

    nc.finalize()
    return nc


def shard_inputs(x, wq, wk, wv, wo, q_norm_w, k_norm_w, cos_table, sin_table,
                 positions, **_ignored):
    """Host-side sharding: returns the list of 8 per-core input maps."""
    x = np.asarray(x, np.float32)
    pos = np.asarray(positions).astype(np.int64)
    cos_sel = np.asarray(cos_table, np.float32)[pos]   # [S, D]
    sin_sel = np.asarray(sin_table, np.float32)[pos]
    qw = np.asarray(q_norm_w, np.float32)
    kw = np.asarray(k_norm_w, np.float32)
    # fold norm weights into the transposed rope tables:
    # w * rope(q') == q'*(w*cos) + rot(q')*(w*sin)
    # also fold rotate-half's minus sign into sin rows 0..63:
    # rope(z) = z*cos + [-z2; z1]*sin = z*cos + [z2; z1]*sin_eff
    sign = np.ones((1, D), np.float32)
    sign[0, :D // 2] = -1.0

    def bf(a):
        return np.ascontiguousarray(a.astype(BF16_NP))

    cosq = bf((cos_sel * qw).T)                        # [D, S]
    sinq = bf((sin_sel * qw * sign).T)
    cosk = bf((cos_sel * kw).T)
    sink = bf((sin_sel * kw * sign).T)
    xTf = x.reshape(S, HID).T                          # [HID, S]
    x_dev = bf(xTf.reshape(NHT, P, S).transpose(1, 0, 2))
    wq = np.asarray(wq, np.float32)
    wk = np.asarray(wk, np.float32)
    wv = np.asarray(wv, np.float32)
    wo = np.asarray(wo, np.float32)

    in_maps = []
    for c in range(N_CORES):
        wqT = wq[c * G * P:(c + 1) * G * P, :].T       # [HID, G*P]
        wkT = wk[c * P:(c + 1) * P, :].T               # [HID, P]
        wvT = wv[c * P:(c + 1) * P, :].T
        woT = wo[:, c * G * P:(c + 1) * G * P].T       # [G*P, HID]
        m = {
            "x_d": x_dev,
            "wq_d": bf(wqT.reshape(NHT, P, G * P).transpose(1, 0, 2)),
            "wk_d": bf(wkT.reshape(NHT, P, P).transpose(1, 0, 2)),
            "wv_d": bf(wvT.reshape(NHT, P, P).transpose(1, 0, 2)),
            "wo_d": bf(woT.reshape(G, P, HID).transpose(1, 0, 2)),
            "cosq": cosq, "sinq": sinq, "cosk": cosk, "sink": sink,
        }
        in_maps.append(m)
    return in_maps


_NC = None


def _get_nc():
    global _NC
    if _NC is None:
        _NC = build_program()
    return _NC


def run_on_device(in_maps, trace=False):
    from concourse.bass_utils import run_bass_kernel_spmd
    nc = _get_nc()
    return run_bass_kernel_spmd(nc, in_maps, list(range(N_CORES)), trace=trace)


def kernel(**inputs):
    in_maps = shard_inputs(**inputs)
    res = run_on_device(in_maps).results
    y = np.zeros((S, HID), np.float32)
    for c in range(N_CORES):
        y += np.asarray(res[c]["y"], np.float32)
    return y.reshape(1, S, HID)


# revision 14
# speedup vs baseline: 1.1544x; 1.1544x over previous
"""GQA attention prefill kernel for Trainium2 (Bass/Tile), 8-way tensor
parallel over heads.

Problem (hardcoded): B=1, S=2048, HID=4096, NH=32, KVH=8, D=128, causal
prefill with per-head RMSNorm on q/k and RoPE, positions = arange(S).

Sharding: core c owns kv-head c and q-heads 4c..4c+3. wq/wo sharded on the
head dim, wk/wv on the kv-head dim; x, rope tables replicated. Each core
computes its 4 heads' contribution through wo; the host sums the 8 partial
outputs.

Device pipeline (all matmul operands bf16, PSUM accumulation fp32):
- weights + rope tables resident in SBUF, loaded once with big DMAs; x
  streamed per 512-column chunk in two halves; y written per 128-row block.
- per chunk: kv projection -> V transpose -> q projection -> rms-norm
  factors via ones-matmul column sums -> rope (rotate-half via SBUF
  partition-swap DMAs) -> causal attention (score/exp/dn/av pipelined
  per k-tile) -> output projection (delayed one chunk so its matmuls fill
  the PE while the next chunk's norm/rope chains run).
- softmax/no-max-subtraction is safe: |scores| <= sqrt(128) after rms-norm.
- reciprocals use the fast DVE Newton approximation (~18 bits, 5x faster
  than the exact one); rms rsqrt = Sqrt activation + fast reciprocal.

Host side: inputs are pre-transposed/pre-reshaped to the exact SBUF layouts
(contraction dim on partitions) and cast to bf16; per-head norm weights and
the rotate-half sign are folded into the rope tables; outputs come back as
bf16 partials summed on host in fp32.
"""

import numpy as np
import ml_dtypes

import concourse.bass as bass
import concourse.mybir as mybir
import concourse.tile as tile
from concourse import bacc
from concourse.masks import make_identity

P = 128
S = 2048
HID = 4096
D = 128
G = 4            # q heads per core
NHT = HID // P   # 32 h-tiles (contraction)
SC = 512         # seq chunk
NSC = S // SC    # 4
NKT = S // P     # 16 k-tiles
EPS = 1e-6
N_CORES = 8

F32 = mybir.dt.float32
BF16 = mybir.dt.bfloat16
BF16_NP = ml_dtypes.bfloat16


def build_program():
    nc = bacc.Bacc("TRN2", target_bir_lowering=False, debug=False)

    # all device tensors are pre-arranged on host to the SBUF layout
    x_d = nc.dram_tensor("x_d", [P, NHT, S], BF16, kind="ExternalInput").ap()
    wq_d = nc.dram_tensor("wq_d", [P, NHT, G * P], BF16, kind="ExternalInput").ap()
    wk_d = nc.dram_tensor("wk_d", [P, NHT, P], BF16, kind="ExternalInput").ap()
    wv_d = nc.dram_tensor("wv_d", [P, NHT, P], BF16, kind="ExternalInput").ap()
    wo_d = nc.dram_tensor("wo_d", [P, G, HID], BF16, kind="ExternalInput").ap()
    cosq = nc.dram_tensor("cosq", [D, S], BF16, kind="ExternalInput").ap()
    sinq = nc.dram_tensor("sinq", [D, S], BF16, kind="ExternalInput").ap()
    cosk = nc.dram_tensor("cosk", [D, S], BF16, kind="ExternalInput").ap()
    sink = nc.dram_tensor("sink", [D, S], BF16, kind="ExternalInput").ap()
    y = nc.dram_tensor("y", [S, HID], BF16, kind="ExternalOutput").ap()

    Sqrt = mybir.ActivationFunctionType.Sqrt
    Exp = mybir.ActivationFunctionType.Exp
    Square = mybir.ActivationFunctionType.Square

    with tile.TileContext(nc) as tc:
        with (
            tc.tile_pool(name="const", bufs=1) as const,
            tc.tile_pool(name="xp", bufs=1) as xp,
            tc.tile_pool(name="scr", bufs=2) as scr,
            tc.tile_pool(name="qrp", bufs=4) as qrp,
            tc.tile_pool(name="ptp", bufs=3) as ptp,
            tc.tile_pool(name="otp", bufs=8) as otp,
            tc.tile_pool(name="ysp", bufs=2) as ysp,
            tc.tile_pool(name="psA", bufs=4, space="PSUM") as psA,
            tc.tile_pool(name="psB", bufs=2, space="PSUM") as psB,
            tc.tile_pool(name="psC", bufs=1, space="PSUM") as psC,
        ):
            # ---- first-chunk x + weight loads, split so the PE can start
            # on the first k/v h-tiles after ~1.5 MB instead of ~12 MB ----
            xa = [None] * NSC
            xb = [None] * NSC
            wk_sb = const.tile([P, NHT, P], BF16)
            wv_sb = const.tile([P, NHT, P], BF16)
            xa[0] = xp.tile([P, NHT // 2, SC], BF16, tag="xa", bufs=2, name="xa0")
            nc.sync.dma_start(xa[0][:, 0:8, :], x_d[:, 0:8, 0:SC])
            nc.sync.dma_start(wk_sb[:, 0:8, :], wk_d[:, 0:8, :])
            nc.sync.dma_start(wv_sb[:, 0:8, :], wv_d[:, 0:8, :])
            nc.sync.dma_start(xa[0][:, 8:16, :], x_d[:, 8:16, 0:SC])
            nc.sync.dma_start(wk_sb[:, 8:16, :], wk_d[:, 8:16, :])
            nc.sync.dma_start(wv_sb[:, 8:16, :], wv_d[:, 8:16, :])
            xb[0] = xp.tile([P, NHT // 2, SC], BF16, tag="xb", bufs=1, name="xb0")
            nc.sync.dma_start(xb[0][:, 0:8, :], x_d[:, 16:24, 0:SC])
            nc.sync.dma_start(wk_sb[:, 16:32, :], wk_d[:, 16:32, :])
            nc.sync.dma_start(wv_sb[:, 16:32, :], wv_d[:, 16:32, :])
            nc.sync.dma_start(xb[0][:, 8:16, :], x_d[:, 24:32, 0:SC])
            wq_sb = const.tile([P, NHT, G * P], BF16)
            for i in range(4):
                nc.sync.dma_start(wq_sb[:, 8 * i:8 * (i + 1), :],
                                  wq_d[:, 8 * i:8 * (i + 1), :])
            ck_sb = const.tile([D, S], BF16)
            nc.sync.dma_start(ck_sb, cosk)
            sk_sb = const.tile([D, S], BF16)
            nc.sync.dma_start(sk_sb, sink)
            cq_sb = const.tile([D, S], BF16)
            nc.sync.dma_start(cq_sb, cosq)
            sq_sb = const.tile([D, S], BF16)
            nc.sync.dma_start(sq_sb, sinq)
            wo_sb = const.tile([P, G, HID], BF16)
            nc.sync.dma_start(wo_sb[:, 0:2, :], wo_d[:, 0:2, :])
            nc.sync.dma_start(wo_sb[:, 2:4, :], wo_d[:, 2:4, :])

            # ---- constants ----
            f32tmp = const.tile([P, SC], F32)
            identity = const.tile([P, P], BF16)
            make_identity(nc, f32tmp[:, 0:P])
            nc.vector.tensor_copy(identity, f32tmp[:, 0:P])
            # ones[k, m] == 1: matmul(out, ones, rhs) -> column sums of rhs
            # broadcast across all 128 output partitions.
            ones_bf = const.tile([P, P], BF16)
            nc.gpsimd.memset(f32tmp, 1.0)
            nc.vector.tensor_copy(ones_bf, f32tmp[:, 0:P])
            # causal masks for the 4 diagonal k-tiles of a q chunk:
            # keep (1.0) where q_local >= 128*j + k_local
            masks = []
            for j in range(4):
                mk = const.tile([P, SC], BF16, name=f"mask{j}")
                nc.gpsimd.memset(f32tmp, 1.0)
                nc.gpsimd.affine_select(
                    f32tmp, f32tmp, pattern=[[1, SC]],
                    compare_op=mybir.AluOpType.is_ge,
                    fill=0.0, base=-P * j, channel_multiplier=-1,
                )
                nc.vector.tensor_copy(mk, f32tmp)
                masks.append(mk)

            bias_keps = const.tile([P, 1], F32)
            nc.gpsimd.memset(bias_keps, float(P) * EPS)
            bias_qeps = const.tile([P, 1], F32)
            nc.gpsimd.memset(bias_qeps, EPS)

            KR = const.tile([P, S], BF16)       # roped+scaled K, [d, s]
            Vs = const.tile([P, NKT, P], BF16)  # V, [s-in-tile, k-tile, d]

            # ots[sc][h]: attention outputs, consumed by the (delayed) o-proj
            ots = [[None] * G for _ in range(NSC)]

            def oproj_gen(sc):
                """output projection for chunk sc, as a generator yielding
                once per matmul so attention(sc+1) can interleave it into
                the PE stream to fill the ACT-exp stalls. Evacuations run
                on DVE (ACT is exp-bound during attention)."""
                q0 = sc * SC
                for stl in range(SC // P):
                    srow = q0 + stl * P
                    for grp in range(2):
                        ys = ysp.tile([P, HID // 2], BF16, tag="ys")
                        yps_l = [psA.tile([P, SC], F32, tag="acc",
                                          name=f"yps{j}") for j in range(4)]
                        for h in range(G):
                            lhs = ots[sc][h][:, stl * P:(stl + 1) * P]
                            for j in range(4):
                                hc = grp * 4 + j
                                nc.tensor.matmul(
                                    yps_l[j], lhs,
                                    wo_sb[:, h, hc * SC:(hc + 1) * SC],
                                    start=(h == 0), stop=(h == G - 1),
                                )
                                yield
                        for j in range(4):
                            if j % 2 == 0:
                                nc.vector.tensor_copy(
                                    ys[:, j * SC:(j + 1) * SC], yps_l[j])
                            else:
                                nc.scalar.copy(
                                    ys[:, j * SC:(j + 1) * SC], yps_l[j])
                        nc.sync.dma_start(
                            y[srow:srow + P,
                              grp * (HID // 2):(grp + 1) * (HID // 2)], ys)

            def drain(gen, n):
                for _ in range(n):
                    try:
                        next(gen)
                    except StopIteration:
                        return

            for sc in range(NSC):
                q0 = sc * SC

                # prefetch next chunk's x
                if sc + 1 < NSC:
                    q1 = (sc + 1) * SC
                    xa[sc + 1] = xp.tile([P, NHT // 2, SC], BF16, tag="xa",
                                         bufs=2, name=f"xa{sc + 1}")
                    nc.sync.dma_start(xa[sc + 1], x_d[:, 0:16, q1:q1 + SC])
                    xb[sc + 1] = xp.tile([P, NHT // 2, SC], BF16, tag="xb",
                                         bufs=1, name=f"xb{sc + 1}")
                    nc.sync.dma_start(xb[sc + 1], x_d[:, 16:32, q1:q1 + SC])

                def xt(ht):
                    return (xa[sc][:, ht, :] if ht < 16
                            else xb[sc][:, ht - 16, :])

                # ---- K/V projections ----
                kps = psA.tile([P, SC], F32, tag="acc")
                vps = psA.tile([P, SC], F32, tag="acc")
                for ht in range(NHT):
                    st, sp = ht == 0, ht == NHT - 1
                    nc.tensor.matmul(kps, wk_sb[:, ht, :], xt(ht),
                                     start=st, stop=sp)
                    nc.tensor.matmul(vps, wv_sb[:, ht, :], xt(ht),
                                     start=st, stop=sp)

                # ---- V: transpose [d, s] -> [s, d] tiles ----
                vtmp = scr.tile([P, SC], BF16, tag="vtmp")
                nc.vector.tensor_copy(vtmp, vps)
                for j in range(SC // P):
                    tp = psB.tile([P, P], BF16, tag="pt", name=f"tp{j}")
                    nc.tensor.transpose(tp, vtmp[:, j * P:(j + 1) * P],
                                        identity)
                    nc.vector.tensor_copy(Vs[:, sc * 4 + j, :], tp)

                # ---- K: sum-of-squares on ACT while q-proj runs on PE ----
                sqk = scr.tile([P, SC], BF16, tag="sqk")
                nc.scalar.activation(sqk, kps, Square)
                kraw = scr.tile([P, SC], BF16, tag="kraw")
                nc.vector.tensor_copy(kraw, kps)

                # ---- Q projections ----
                qps = [psA.tile([P, SC], F32, tag="acc", name=f"qps{i}")
                       for i in range(G)]
                for ht in range(NHT):
                    st, sp = ht == 0, ht == NHT - 1
                    for mt in range(G):
                        nc.tensor.matmul(
                            qps[mt], wq_sb[:, ht, mt * P:(mt + 1) * P],
                            xt(ht), start=st, stop=sp,
                        )

                # ---- K: norm factor + rope -> KR columns ----
                ssb = psB.tile([P, SC], F32, tag="pt")
                nc.tensor.matmul(ssb, ones_bf, sqk, start=True, stop=True)
                # 1/sqrt(ssq + d*eps) == rsqrt(mean+eps)/sqrt(d):
                # k-norm and softmax 1/sqrt(d) in one factor
                rkf = scr.tile([P, SC], F32, tag="rkf")
                nc.scalar.activation(rkf, ssb, Sqrt, bias=bias_keps, scale=1.0)
                rkr = scr.tile([P, SC], F32, tag="rkr")
                nc.vector.reciprocal_approx_fast(rkr, rkf)
                # rope: out = z*cos + rot(z)*sin_eff, rot(z) = [z2; z1]
                # (rotate-half minus sign folded into sin_eff on host)
                krot = scr.tile([P, SC], BF16, tag="krot")
                nc.sync.dma_start(krot[0:64], kraw[64:128])
                nc.sync.dma_start(krot[64:128], kraw[0:64])
                t1 = scr.tile([P, SC], BF16, tag="t1")
                nc.vector.tensor_mul(t1, krot, sk_sb[:, q0:q0 + SC])
                kpre = scr.tile([P, SC], BF16, tag="kpre")
                nc.vector.tensor_mul(kpre, kraw, ck_sb[:, q0:q0 + SC])
                nc.vector.tensor_add(kpre, kpre, t1)
                nc.vector.tensor_mul(KR[:, q0:q0 + SC], kpre, rkr)

                # ---- Q per head: norm factor + rope ----
                qrs = []
                for h in range(G):
                    sqq = scr.tile([P, SC], BF16, tag="sqk")
                    nc.scalar.activation(sqq, qps[h], Square)
                    ssbq = psB.tile([P, SC], F32, tag="pt")
                    nc.tensor.matmul(ssbq, ones_bf, sqq,
                                     start=True, stop=True)
                    rqf = scr.tile([P, SC], F32, tag="rkf")
                    nc.scalar.activation(rqf, ssbq, Sqrt,
                                         bias=bias_qeps, scale=1.0 / P)
                    rqr = scr.tile([P, SC], F32, tag="rkr")
                    nc.vector.reciprocal_approx_fast(rqr, rqf)
                    qraw = scr.tile([P, SC], BF16, tag="kraw")
                    nc.vector.tensor_copy(qraw, qps[h])
                    qrot = scr.tile([P, SC], BF16, tag="krot")
                    nc.sync.dma_start(qrot[0:64], qraw[64:128])
                    nc.sync.dma_start(qrot[64:128], qraw[0:64])
                    t1b = scr.tile([P, SC], BF16, tag="t1")
                    nc.vector.tensor_mul(t1b, qrot, sq_sb[:, q0:q0 + SC])
                    qpre = scr.tile([P, SC], BF16, tag="kpre")
                    nc.vector.tensor_mul(qpre, qraw, cq_sb[:, q0:q0 + SC])
                    nc.vector.tensor_add(qpre, qpre, t1b)
                    qr = qrp.tile([P, SC], BF16, tag="qr")
                    nc.vector.tensor_mul(qr, qpre, rqr)
                    qrs.append(qr)

                # ---- attention; o-proj of the previous chunk interleaves
                # into the PE stream to cover the ACT exp latency ----
                filler = oproj_gen(sc - 1) if sc > 0 else iter(())
                nkt = (sc + 1) * 4

                def qoff(kt):
                    # diagonal k-tile j attends only to q >= j*128 (causal)
                    return max(0, kt - sc * 4) * P

                for h in range(G):
                    avp = psC.tile([P, SC], F32, tag="av")
                    acc = scr.tile([P, SC], BF16, tag="acc")
                    ptps = [None] * nkt
                    ptps[0] = psB.tile([P, SC], F32, tag="pt", name="ptps0")
                    nc.tensor.matmul(ptps[0], KR[:, 0:P], qrs[h][:, qoff(0):],
                                     start=True, stop=True)
                    for kt in range(nkt):
                        o = qoff(kt)
                        if kt + 1 < nkt:
                            k1 = (kt + 1) * P
                            o1 = qoff(kt + 1)
                            ptps[kt + 1] = psB.tile([P, SC], F32, tag="pt",
                                                    name=f"ptps{kt + 1}")
                            nc.tensor.matmul(ptps[kt + 1][:, o1:],
                                             KR[:, k1:k1 + P],
                                             qrs[h][:, o1:],
                                             start=True, stop=True)
                        pt = ptp.tile([P, SC], BF16, tag="pt")
                        nc.scalar.activation(pt[:, o:], ptps[kt][:, o:], Exp)
                        if kt >= sc * 4:
                            # triangle mask on the partial 128-wide block
                            nc.vector.tensor_mul(pt[:, o:o + P], pt[:, o:o + P],
                                                 masks[0][:, 0:P])
                        # denominator: cheap DVE accumulate, one matmul
                        # (for the cross-partition sum + broadcast) at end
                        if kt == 0:
                            nc.vector.tensor_copy(acc, pt)
                        else:
                            nc.vector.tensor_add(acc[:, o:], acc[:, o:],
                                                 pt[:, o:])
                        nc.tensor.matmul(avp[:, o:], Vs[:, kt, :], pt[:, o:],
                                         start=(kt == 0), stop=(kt == nkt - 1))
                        drain(filler, 3)
                    dnp = psC.tile([P, SC], F32, tag="dn")
                    nc.tensor.matmul(dnp, ones_bf, acc, start=True, stop=True)
                    rcp = scr.tile([P, SC], F32, tag="rcp")
                    nc.vector.reciprocal_approx_fast(rcp, dnp)
                    ot = otp.tile([P, SC], BF16, tag="ot")
                    nc.vector.tensor_mul(ot, avp, rcp)
                    ots[sc][h] = ot
                drain(filler, 10 ** 6)

            drain(oproj_gen(NSC - 1), 10 ** 6)

    nc.finalize()
    return nc


def shard_inputs(x, wq, wk, wv, wo, q_norm_w, k_norm_w, cos_table, sin_table,
                 positions, **_ignored):
    """Host-side sharding: returns the list of 8 per-core input maps."""
    x = np.asarray(x, np.float32)
    pos = np.asarray(positions).astype(np.int64)
    cos_sel = np.asarray(cos_table, np.float32)[pos]   # [S, D]
    sin_sel = np.asarray(sin_table, np.float32)[pos]
    qw = np.asarray(q_norm_w, np.float32)
    kw = np.asarray(k_norm_w, np.float32)
    # fold norm weights into the transposed rope tables:
    # w * rope(q') == q'*(w*cos) + rot(q')*(w*sin)
    # also fold rotate-half's minus sign into sin rows 0..63:
    # rope(z) = z*cos + [-z2; z1]*sin = z*cos + [z2; z1]*sin_eff
    sign = np.ones((1, D), np.float32)
    sign[0, :D // 2] = -1.0

    def bf(a):
        return np.ascontiguousarray(a.astype(BF16_NP))

    cosq = bf((cos_sel * qw).T)                        # [D, S]
    sinq = bf((sin_sel * qw * sign).T)
    cosk = bf((cos_sel * kw).T)
    sink = bf((sin_sel * kw * sign).T)
    xTf = x.reshape(S, HID).T                          # [HID, S]
    x_dev = bf(xTf.reshape(NHT, P, S).transpose(1, 0, 2))
    wq = np.asarray(wq, np.float32)
    wk = np.asarray(wk, np.float32)
    wv = np.asarray(wv, np.float32)
    wo = np.asarray(wo, np.float32)

    in_maps = []
    for c in range(N_CORES):
        wqT = wq[c * G * P:(c + 1) * G * P, :].T       # [HID, G*P]
        wkT = wk[c * P:(c + 1) * P, :].T               # [HID, P]
        wvT = wv[c * P:(c + 1) * P, :].T
        woT = wo[:, c * G * P:(c + 1) * G * P].T       # [G*P, HID]
        m = {
            "x_d": x_dev,
            "wq_d": bf(wqT.reshape(NHT, P, G * P).transpose(1, 0, 2)),
            "wk_d": bf(wkT.reshape(NHT, P, P).transpose(1, 0, 2)),
            "wv_d": bf(wvT.reshape(NHT, P, P).transpose(1, 0, 2)),
            "wo_d": bf(woT.reshape(G, P, HID).transpose(1, 0, 2)),
            "cosq": cosq, "sinq": sinq, "cosk": cosk, "sink": sink,
        }
        in_maps.append(m)
    return in_maps


_NC = None


def _get_nc():
    global _NC
    if _NC is None:
        _NC = build_program()
    return _NC


def run_on_device(in_maps, trace=False):
    from concourse.bass_utils import run_bass_kernel_spmd
    nc = _get_nc()
    return run_bass_kernel_spmd(nc, in_maps, list(range(N_CORES)), trace=trace)


def kernel(**inputs):
    in_maps = shard_inputs(**inputs)
    res = run_on_device(in_maps).results
    y = np.zeros((S, HID), np.float32)
    for c in range(N_CORES):
        y += np.asarray(res[c]["y"], np.float32)
    return y.reshape(1, S, HID)
